# revision 6
# baseline (speedup 1.0000x reference)
"""GCN SpMM kernel for Trainium2 (8 NeuronCores, SPMD) — batched-gather version.

Computation (see reference):
    h = x @ W.T
    deg[n] = #edges with col==n;  dis = rsqrt(deg) (0 where deg==0)
    norm_e = dis[row_e] * dis[col_e]
    out[r] = sum_{e: row_e==r} norm_e * h[col_e]

Linearity: aggregate x first, project once per output tile:
    out = (segment_sum(norm_e * x[col_e], row_e)) @ W.T

Device strategy (per core; output rows sharded 8 ways, x replicated):
  - edges sorted by (superblock of SB dest-windows, source-shard, window);
    per-(sb, shard) groups padded to a multiple of 128 edges shared across
    cores (sizes = max over cores), pad edges gather row 0 with norm 0.
  - one InstDMAGatherAnt per (sb, shard): thousands of rows per instruction
    (the previous per-chunk indirect DMA paid a ~1us SWDGE fixed cost per
    128 rows — Pool engine was 98% busy on that alone).  Source indices are
    int16, so x is split into 4 row-shards of 25000 and the gather base AP
    selects the shard.
  - per 128-edge chunk, one DVE op builds the scatter matrix
        S[e, j*128+d] = (iota[j*128+d] == dstrel_e) * norm_e
    covering every window the chunk straddles (dstrel is window-relative to
    the chunk's first window; the iota compare zero-masks foreign edges).
  - PE accumulates agg[f, d] += G_c[e, f]^T @ S_c[e, d] into [128, 512]
    fp32 PSUM banks (4 dest windows per bank).  The first matmul touching a
    bank is widened to the full bank and uses start=True to zero it.
  - Act engine evacuates each finished bank to fp16 SBUF, PE applies W
    (bank-wide matmul), Act copies the result into the fp16 output tile;
    one DMA stores it at the end.

All device compute is fp16 (x, S, W) with fp32 PSUM accumulation; the
host up-casts the output to fp32.  Relative error vs the fp32 reference is
~1e-3, well inside the 2e-2 gate.
"""

import sys

sys.path.insert(0, "/opt/trn_rl_repo")

import numpy as np

import concourse.bass as bass
import concourse.mybir as mybir
import concourse.tile as tile
from concourse import library_config
from concourse.alu_op_type import AluOpType
from concourse.bass_utils import run_bass_kernel_spmd
from concourse.library_overlay import lower_extended_insts
from concourse.vector_clock import ScopedClock

# ---------------------------------------------------------------------------
# Workaround: this walrus build rejects instructions with >1 sync wait
# ("Too many sync wait commands"). TileContext's kernel-tail drain can carry
# several; split them across multiple drain instructions.
_MAX_WAITS = 1


def _split_drain_and_barrier(self, tick_clock, wait_clock):
    nc = self.nc
    drain_inst = nc.sync.drain()
    wait_clock.add_sem_waits(
        drain_inst.ins, ScopedClock({None: tick_clock.global_clock})
    )
    si = drain_inst.ins.sync_info
    if si is not None and si.on_wait and len(si.on_wait) > _MAX_WAITS:
        waits = list(si.on_wait)
        si.on_wait = waits[:_MAX_WAITS]
        rest = waits[_MAX_WAITS:]
        while rest:
            d2 = nc.sync.drain()
            si2 = d2.ins.sync_info
            if si2 is None:
                d2.ins.sync_info = mybir.SyncInfo(
                    on_wait=rest[:_MAX_WAITS], on_update=[]
                )
            else:
                si2.on_wait = rest[:_MAX_WAITS]
            rest = rest[_MAX_WAITS:]
    nc.all_engine_barrier()
    assert self.sems is not None
    popped = nc._tile_sem_poison_stack.pop()
    assert popped is self._sem_poison
    nc.clear_and_free_semaphores(list(self.sems.allocated().values()))
    nc.all_engine_barrier()


tile.TileContext._drain_and_barrier = _split_drain_and_barrier


def split_multi_waits(nc):
    """Split any instruction carrying >1 sem waits: move extra waits onto
    same-engine NOPs inserted immediately before it (engines execute their
    instructions in block order, so this is equivalent)."""
    n = 0
    for bb in nc.m.functions[0].blocks:
        new_insts = []
        for ins in bb.instructions:
            si = ins.sync_info
            if si is not None and si.on_wait and len(si.on_wait) > 1:
                waits = list(si.on_wait)
                for w in waits[:-1]:
                    n += 1
                    nop = mybir.InstNoOp(
                        name=f"waitsplit-{n}-{ins.name}",
                        sync_info=mybir.SyncInfo(on_wait=[w], on_update=[]),
                        bass_nofuse=True,
                        engine=ins.engine,
                    )
                    new_insts.append(nop)
                si.on_wait = waits[-1:]
            new_insts.append(ins)
        bb.instructions[:] = new_insts
    return n


# ---------------------------------------------------------------------------

N_NODES = 100000
F = 128
N_CORES = 8
ROWS_PER_CORE = N_NODES // N_CORES  # 12500
WIN = 128  # dest rows per window
N_WIN = (ROWS_PER_CORE + WIN - 1) // WIN  # 98
SB = 14  # windows per superblock
N_SB = N_WIN // SB  # 14
assert N_SB * SB == N_WIN
NSHARD = 4  # x row-shards (int16 gather index limit)
SHARD_ROWS = (N_NODES + NSHARD - 1) // NSHARD  # 25000
BANK_WINS = 4  # dest windows per PSUM agg bank
MAXSPAN = 4  # max windows one 128-edge chunk may straddle
GCAP = 1024  # max indices per dma_gather (SWDGE descriptor-ring capacity)
IOTA_W = (MAXSPAN + BANK_WINS - 1) * WIN  # 896: first-touch widened S


def host_prep(x, W, edge_index):
    """Compute norms, bucket/sort/pad edges, build the shared schedule and
    the per-core device-layout arrays."""
    row = np.asarray(edge_index[0]).astype(np.int64)
    col = np.asarray(edge_index[1]).astype(np.int64)
    x = np.asarray(x, dtype=np.float32)
    W = np.asarray(W, dtype=np.float32)
    E = row.shape[0]

    deg = np.bincount(col, minlength=N_NODES).astype(np.float64)
    with np.errstate(divide="ignore"):
        dis = np.where(deg > 0, 1.0 / np.sqrt(deg), 0.0).astype(np.float32)
    norm = dis[row] * dis[col]

    x16 = np.ascontiguousarray(x.astype(np.float16))
    wt16 = np.ascontiguousarray(W.T.astype(np.float16))
    iota = np.tile(np.arange(IOTA_W, dtype=np.float16), (128, 1))

    NG = N_SB * NSHARD  # groups per core

    # -- per-core bucketing ------------------------------------------------
    cores = []
    gsize = np.zeros((N_CORES, NG), dtype=np.int64)
    for k in range(N_CORES):
        m = (row >= k * ROWS_PER_CORE) & (row < (k + 1) * ROWS_PER_CORE)
        r = row[m] - k * ROWS_PER_CORE
        c = col[m]
        nm = norm[m]
        win = r >> 7
        shard = c // SHARD_ROWS
        grp = (win // SB) * NSHARD + shard
        order = np.lexsort((win, grp))
        r, c, nm, win, grp = r[order], c[order], nm[order], win[order], grp[order]
        gsize[k] = np.bincount(grp, minlength=NG)
        cores.append((r, c, nm, win, grp))

    # shared padded group sizes: dma_gather only needs 16-granular counts;
    # the final partial chunk of each group runs with zeroed garbage rows
    gpad = ((gsize.max(axis=0) + 15) // 16) * 16
    gch = (gpad + 127) // 128  # chunks per group
    ch0 = np.concatenate([[0], np.cumsum(gch)])  # first chunk of group
    icol16 = np.concatenate([[0], np.cumsum(gpad // 16)])  # idx col offsets
    total = int(gpad.sum())  # 16-granular slot count (= descriptors)
    n_chunk = int(gch.sum())

    # -- per-core slot data ------------------------------------------------
    idx_all = np.zeros((N_CORES, 32, total // 16), dtype=np.int16)
    dst_all = np.full((N_CORES, 128, n_chunk), -1.0, dtype=np.float32)
    nrm_all = np.zeros((N_CORES, 128, n_chunk), dtype=np.float32)
    wfirst = np.full((N_CORES, n_chunk), 1 << 30, dtype=np.int64)
    wlast = np.full((N_CORES, n_chunk), -1, dtype=np.int64)

    for k in range(N_CORES):
        r, c, nm, win, grp = cores[k]
        gst_real = np.concatenate([[0], np.cumsum(gsize[k])])
        pos = np.arange(len(r)) - gst_real[grp]  # slot within group
        ch = ch0[grp] + pos // 128  # global chunk id
        part = pos % 128  # partition slot within chunk

        np.minimum.at(wfirst[k], ch, win)
        np.maximum.at(wlast[k], ch, win)

        cl = np.zeros(total, dtype=np.int16)
        cl[icol16[grp] * 16 + pos] = (c - (grp % NSHARD) * SHARD_ROWS).astype(
            np.int16
        )
        wrap = cl.reshape(-1, 16).T  # [16, total/16]: idx i at (i%16, i//16)
        idx_all[k, 0:16] = wrap
        idx_all[k, 16:32] = wrap

        nrm_all[k][part, ch] = nm

    # shared per-chunk window range (union over cores)
    wf = wfirst.min(axis=0)
    wl = wlast.max(axis=0)
    assert (wl >= 0).all(), "empty chunk (group padded from zero edges?)"
    span = wl - wf + 1
    assert (span <= MAXSPAN).all(), f"chunk straddles {span.max()} windows"

    for k in range(N_CORES):
        r, c, nm, win, grp = cores[k]
        gst_real = np.concatenate([[0], np.cumsum(gsize[k])])
        pos = np.arange(len(r)) - gst_real[grp]
        ch = ch0[grp] + pos // 128
        dst_all[k][pos % 128, ch] = (
            (r & (WIN - 1)) + WIN * (win - wf[ch])
        ).astype(np.float32)

    # -- shared schedule ---------------------------------------------------
    # per (sb): chunk range, per-shard gather (idx col range, chunk range),
    # per chunk: matmul list [(bank, s_off_cols, out_off_cols, len_cols,
    #                          start, stop, s_width)]
    sched = []
    for sb in range(N_SB):
        g0 = sb * NSHARD
        ch_lo = int(ch0[g0])
        ch_hi = int(ch0[g0 + NSHARD])
        gathers = []
        for s in range(NSHARD):
            n_idx = int(gpad[g0 + s])
            if n_idx == 0:
                continue
            gathers.append(
                dict(
                    shard=s,
                    n_idx=n_idx,
                    icol0=int(icol16[g0 + s]),
                    ch0=int(ch0[g0 + s]),
                    n_ch=int(gch[g0 + s]),
                )
            )
        # banks of this sb: windows [sb*SB + b*BANK_WINS, ...)
        w_lo = sb * SB
        n_banks = (SB + BANK_WINS - 1) // BANK_WINS
        bank_lo = [w_lo + b * BANK_WINS for b in range(n_banks)]
        bank_hi = [min(w_lo + (b + 1) * BANK_WINS, w_lo + SB) for b in range(n_banks)]
        touched = [False] * n_banks
        mm = {ch: [] for ch in range(ch_lo, ch_hi)}
        last_mm = [None] * n_banks
        for ch in range(ch_lo, ch_hi):
            cw_lo, cw_hi = int(wf[ch]), int(wl[ch]) + 1
            s_width = (cw_hi - cw_lo) * WIN
            for b in range(n_banks):
                i_lo = max(cw_lo, bank_lo[b])
                i_hi = min(cw_hi, bank_hi[b])
                if i_lo >= i_hi:
                    continue
                if not touched[b]:
                    # widen to the full bank and zero it (start=True)
                    assert cw_lo <= bank_lo[b], (
                        f"first touch of bank {b} (sb {sb}) has wf {cw_lo} > "
                        f"bank_lo {bank_lo[b]}"
                    )
                    i_lo, i_hi = bank_lo[b], bank_hi[b]
                    s_width = max(s_width, (i_hi - cw_lo) * WIN)
                    start = True
                    touched[b] = True
                else:
                    start = False
                e = dict(
                    bank=b,
                    s_off=(i_lo - cw_lo) * WIN,
                    out_off=(i_lo - bank_lo[b]) * WIN,
                    ncol=(i_hi - i_lo) * WIN,
                    start=start,
                    stop=False,
                )
                mm[ch].append(e)
                last_mm[b] = e
            mm[ch] = mm[ch]
            # record final s width
            if mm[ch]:
                mm[ch][0]["s_width"] = max(
                    max(e["s_off"] + e["ncol"] for e in mm[ch]), s_width
                )
        assert all(touched), f"sb {sb}: bank never touched"
        for e in last_mm:
            e["stop"] = True
        sched.append(
            dict(
                sb=sb,
                ch_lo=ch_lo,
                ch_hi=ch_hi,
                gathers=gathers,
                n_banks=n_banks,
                bank_cols=[(bank_hi[b] - bank_lo[b]) * WIN for b in range(n_banks)],
                bank_w0=[bank_lo[b] for b in range(n_banks)],
                mm=mm,
            )
        )

    pad_pct = (total * N_CORES / E - 1) * 100
    stats = dict(total=total, n_chunk=n_chunk, pad_pct=pad_pct)
    return x16, wt16, iota, idx_all, dst_all, nrm_all, sched, stats


def build_program(sched, n_chunk, total, split_waits=True):
    """Build the per-core Bass/Tile program (identical on all cores)."""
    nc = bass.Bass("TRN2", target_bir_lowering=False, debug=False, num_devices=1)
    dt = mybir.dt

    x_d = nc.dram_tensor("x", [N_NODES, F], dt.float16, kind="ExternalInput")
    wt_d = nc.dram_tensor("wt", [F, F], dt.float16, kind="ExternalInput")
    iota_d = nc.dram_tensor("iota", [128, IOTA_W], dt.float16, kind="ExternalInput")
    idx_d = nc.dram_tensor("idx", [32, total // 16], dt.int16, kind="ExternalInput")
    dst_d = nc.dram_tensor("dst", [128, n_chunk], dt.float32, kind="ExternalInput")
    nrm_d = nc.dram_tensor("nrm", [128, n_chunk], dt.float32, kind="ExternalInput")
    y_d = nc.dram_tensor("y", [F, N_WIN * WIN], dt.float16, kind="ExternalOutput")

    with tile.TileContext(nc) as tc:
        with (
            tc.tile_pool(name="const", bufs=1) as const_pool,
            tc.tile_pool(name="out", bufs=1) as out_pool,
            tc.tile_pool(name="gather", bufs=2) as g_pool,
            tc.tile_pool(name="meta", bufs=1) as meta_pool,
            tc.tile_pool(name="s", bufs=8) as s_pool,
            tc.tile_pool(name="aggsb", bufs=3) as asb_pool,
            tc.tile_pool(name="psum_agg", bufs=1, space="PSUM") as pa_pool,
            tc.tile_pool(name="psum_proj", bufs=2, space="PSUM") as pp_pool,
        ):
            nc.gpsimd.load_library(library_config.mlp)
            nidx_reg = nc.gpsimd.alloc_register("nidx")
            # consts ride the Act HWDGE queue so the SP queue can start
            # streaming the first superblock's indices immediately
            wt_sb = const_pool.tile([F, F], dt.float16, tag="wt")
            nc.scalar.dma_start(out=wt_sb[:], in_=wt_d.ap())
            iota_sb = const_pool.tile([128, IOTA_W], dt.float16, tag="iota")
            nc.scalar.dma_start(out=iota_sb[:], in_=iota_d.ap())
            out_sb = out_pool.tile([F, N_WIN * WIN], dt.float16, tag="out")

            # all metadata prefetched in three bulk DMAs: the first
            # superblock's slice lands first so gathers start immediately
            icolT = total // 16
            idx_t = meta_pool.tile([32, icolT], dt.int16, tag="idx")
            icolA = sched[0]["gathers"][-1]["icol0"] + sched[0]["gathers"][-1]["n_idx"] // 16
            nc.sync.dma_start(out=idx_t[:, :icolA], in_=idx_d.ap()[:, :icolA])
            nc.sync.dma_start(out=idx_t[:, icolA:], in_=idx_d.ap()[:, icolA:])
            dst_t = meta_pool.tile([128, n_chunk], dt.float32, tag="dst")
            nc.scalar.dma_start(out=dst_t[:], in_=dst_d.ap())
            nrm_t = meta_pool.tile([128, n_chunk], dt.float32, tag="nrm")
            nc.scalar.dma_start(out=nrm_t[:], in_=nrm_d.ap())

            for blk in sched:
                ch_lo, ch_hi = blk["ch_lo"], blk["ch_hi"]
                icol0 = 0

                # batched gathers, one per source shard, split to the SWDGE
                # descriptor-ring capacity; every non-final piece is a whole
                # number of 128-row chunks so descriptor placement aligns
                g_ts = {}
                first_use = blk["sb"] < 2  # g_pool bufs=2: sb0/sb1 are fresh
                for g in blk["gathers"]:
                    s = g["shard"]
                    g_t = g_pool.tile([128, g["n_ch"], F], dt.float16, tag=f"g{s}")
                    if first_use and g["n_idx"] % 128:
                        # zero the partial tail chunk once: its garbage rows
                        # must be finite (S rows there are 0; 0*NaN = NaN)
                        nc.vector.memset(g_t[:, g["n_ch"] - 1, :], 0)
                    lo = s * SHARD_ROWS
                    hi = min(lo + SHARD_ROWS, N_NODES)
                    ic0 = g["icol0"]
                    # on the very last group, taper the final piece so the
                    # end-of-kernel drain only waits on a short gather
                    last_grp = blk is sched[-1] and g is blk["gathers"][-1]
                    pieces = []
                    o = 0
                    while o < g["n_idx"]:
                        n = min(GCAP, g["n_idx"] - o)
                        pieces.append((o, n))
                        o += n
                    if last_grp and pieces and pieces[-1][1] >= 384:
                        o0, n0 = pieces.pop()
                        tail = 128 + n0 % 128
                        pieces.append((o0, n0 - tail))
                        pieces.append((o0 + n0 - tail, tail))
                    for o, n in pieces:
                        nc.gpsimd.reg_mov(nidx_reg, n)
                        nc.gpsimd.dma_gather(
                            g_t[:, o // 128 : -(-(o + n) // 128), :],
                            x_d.ap()[lo:hi, :],
                            idx_t[:, ic0 + o // 16 : ic0 + (o + n) // 16],
                            n,
                            nidx_reg,
                            F,
                        )
                    g_ts[s] = (g_t, g["ch0"])

                # PSUM accumulator banks
                aggs = []
                for b in range(blk["n_banks"]):
                    aggs.append(
                        pa_pool.tile(
                            [128, 512], dt.float32, tag=f"aggb{b}", name=f"aggb{b}"
                        )
                    )

                # matmuls in chunk order
                for g in blk["gathers"]:
                    s = g["shard"]
                    g_t, ch0 = g_ts[s]
                    for ci in range(g["n_ch"]):
                        ch = ch0 + ci
                        mms = blk["mm"][ch]
                        if not mms:
                            continue
                        s_w = mms[0]["s_width"]
                        s_t = s_pool.tile([128, IOTA_W], dt.float16, tag="s")
                        nc.vector.tensor_scalar(
                            out=s_t[:, :s_w],
                            in0=iota_sb[:, :s_w],
                            scalar1=dst_t[:, ch : ch + 1],
                            scalar2=nrm_t[:, ch : ch + 1],
                            op0=AluOpType.is_equal,
                            op1=AluOpType.mult,
                        )
                        for e in mms:
                            nc.tensor.matmul(
                                aggs[e["bank"]][:, e["out_off"] : e["out_off"] + e["ncol"]],
                                lhsT=g_t[:, ci, :],
                                rhs=s_t[:, e["s_off"] : e["s_off"] + e["ncol"]],
                                start=e["start"],
                                stop=e["stop"],
                                skip_group_check=True,
                            )
                            if e["stop"]:
                                b = e["bank"]
                                bc = blk["bank_cols"][b]
                                w0 = blk["bank_w0"][b]
                                agg_sb = asb_pool.tile(
                                    [128, 512], dt.float16, tag="aggsb"
                                )
                                nc.scalar.copy(
                                    out=agg_sb[:, :bc], in_=aggs[b][:, :bc]
                                )
                                proj = pp_pool.tile(
                                    [128, 512], dt.float32, tag="proj"
                                )
                                nc.tensor.matmul(
                                    proj[:, :bc],
                                    lhsT=wt_sb[:],
                                    rhs=agg_sb[:, :bc],
                                    start=True,
                                    stop=True,
                                )
                                nc.scalar.copy(
                                    out=out_sb[:, w0 * WIN : w0 * WIN + bc],
                                    in_=proj[:, :bc],
                                )
                                nc.sync.dma_start(
                                    out=y_d.ap()[:, w0 * WIN : w0 * WIN + bc],
                                    in_=out_sb[:, w0 * WIN : w0 * WIN + bc],
                                )
    if split_waits:
        split_multi_waits(nc)
    lower_extended_insts(nc)
    return nc


def kernel(x, W, edge_index):
    x16, wt16, iota, idx_all, dst_all, nrm_all, sched, stats = host_prep(
        x, W, edge_index
    )
    nc = build_program(sched, stats["n_chunk"], stats["total"])
    in_maps = [
        {
            "x": x16,
            "wt": wt16,
            "iota": iota,
            "idx": idx_all[k],
            "dst": dst_all[k],
            "nrm": nrm_all[k],
        }
        for k in range(N_CORES)
    ]
    res = run_bass_kernel_spmd(nc, in_maps, core_ids=list(range(N_CORES)))
    outs = []
    for k in range(N_CORES):
        y_t = res.results[k]["y"]  # [F, N_WIN*WIN] fp16, feature-major
        outs.append(np.ascontiguousarray(y_t[:, :ROWS_PER_CORE].T.astype(np.float32)))
    return np.concatenate(outs, axis=0)
